# revision 1
# baseline (speedup 1.0000x reference)
"""AFT-simple attention (nn_AsfAttention) on 8 TRN2 NeuronCores.

Reference (per batch b):
    emb_q = q @ w_q; emb_k = k @ w_k; emb_v = v @ w_v
    k_exp = exp(emb_k)
    y = sigmoid(emb_q) * cumsum(k_exp * emb_v, seq) / cumsum(k_exp, seq)
    out = y @ w_p

Sharding: core c = 2*b + h handles batch b, seq half h (4096 rows each).
The cumsum carry across the half boundary is a [H]-vector pair exchanged
via an all-8 AllReduce of a slot-masked buffer (pairwise replica groups
crash the runtime here; slots cost nothing at this size).

Layout: everything channel-major on chip ([ch partitions, seq free]) so
  - projections use the weights directly as the stationary operand
    (lhsT = W[ch_in, ch_out], rhs = x^T[ch_in, seq]),
  - the seq cumsum is a native tensor_tensor_scan along the free dim,
  - the output projection takes y^T tiles as lhsT and yields natural
    [seq, ch_out] ready for DMA out.
Host pre-transposes q/k/v shards to [H, 4096] and casts to bf16; matmuls
run bf16 with fp32 PSUM accumulation, scans keep fp32 state.
"""

import sys

import numpy as np
import ml_dtypes

import concourse.bass as bass
import concourse.tile as tile
from concourse import bacc, mybir
from concourse.bass_utils import run_bass_kernel_spmd

B, S, H = 4, 8192, 1024
NCORES = 8
SC_CORE = S // 2          # 4096 seq rows per core
SC = 512                  # seq chunk (columns per matmul / scan step)
NS = SC_CORE // SC        # 8 seq chunks
M = H // 128              # 8 ch_out tiles
K = H // 128              # 8 ch_in tiles
NT = SC // 128            # 4 seq subtiles per chunk for the out projection

bf16 = mybir.dt.bfloat16
f32 = mybir.dt.float32
AF = mybir.ActivationFunctionType
OP = mybir.AluOpType

_cache = {}


def prune_pe_incs(nc, verbose=False):
    """Drop the per-matmul PE semaphore increment from every matmul that
    is not the end of its accumulation group (stop_tensor_calc), then
    renumber all waits on that semaphore. A wait whose original target
    inc was dropped is rounded UP to the next kept inc — safe here
    because kept-inc matmuls never depend on the rounded-up waiters
    (they only consume earlier-generation tiles).

    Each inc is a serialized EVT_SEM register write (~26 ns) on the PE
    sequencer; at 2048 matmuls this is ~45 us of PE issue overhead.
    """
    import bisect
    from collections import defaultdict

    insts = []
    for bb in nc.main_func.blocks:
        insts.extend(bb.instructions)

    upd = defaultdict(list)
    for pos, ins in enumerate(insts):
        si = ins.sync_info
        if not si:
            continue
        for u in si.on_update:
            upd[u.ant_name].append((pos, ins, u))

    changed = 0
    for sem, us in upd.items():
        if len(us) < 64:
            continue
        if not all(type(i).__name__ == "InstMatmult"
                   and u.update_mode == "sem-inc" and u.update_value == 1
                   for _, i, u in us):
            continue
        kept = [bool(i.stop_tensor_calc) for _, i, _ in us]
        kept[-1] = True
        kept_idx = [j for j, k in enumerate(kept) if k]

        def new_thresh(t):
            j = bisect.bisect_left(kept_idx, t - 1)
            assert j < len(kept_idx), f"wait {t} beyond last kept inc"
            return j + 1

        for ins in insts:
            si = ins.sync_info
            if not si:
                continue
            for w in si.on_wait:
                if w.ant_name == sem:
                    assert w.wait_mode == "sem-ge-imm", w.wait_mode
                    w.wait_value = new_thresh(w.wait_value)
        for j, (_, ins, u) in enumerate(us):
            if not kept[j]:
                si = ins.sync_info
                rest = [x for x in si.on_update if x.ant_name != sem]
                assert len(rest) == len(si.on_update) - 1
                if rest:
                    si.on_update[:] = rest
                else:
                    si.on_update.clear()
                changed += 1
    if verbose:
        print(f"prune_pe_incs: removed {changed} matmul sem-incs")
    return changed


def build(ns=NS, debug=False):
    sc_core = ns * SC
    nc = bacc.Bacc("TRN2", target_bir_lowering=False, debug=debug,
                   num_devices=NCORES)

    qT_e = nc.dram_tensor("qT", [H, sc_core], bf16, kind="ExternalInput")
    kT_e = nc.dram_tensor("kT", [H, sc_core], bf16, kind="ExternalInput")
    vT_e = nc.dram_tensor("vT", [H, sc_core], bf16, kind="ExternalInput")
    wq_e = nc.dram_tensor("wq", [H, H], bf16, kind="ExternalInput")
    wk_e = nc.dram_tensor("wk", [H, H], bf16, kind="ExternalInput")
    wv_e = nc.dram_tensor("wv", [H, H], bf16, kind="ExternalInput")
    wp_e = nc.dram_tensor("wp", [H, H], bf16, kind="ExternalInput")
    # slot masks for the carry exchange: totals live in a [128, 8*16]
    # buffer (8 core slots x (2 quantities x 8 m-tiles)).
    sself_e = nc.dram_tensor("slot_self", [128, NCORES * 2 * M], f32,
                             kind="ExternalInput")
    spart_e = nc.dram_tensor("slot_partner", [128, NCORES * 2 * M], f32,
                             kind="ExternalInput")
    out_e = nc.dram_tensor("out", [sc_core, H], f32, kind="ExternalOutput")

    sk_sp = nc.dram_tensor("sk_sp", [H, sc_core], bf16)
    skv_sp = nc.dram_tensor("skv_sp", [H, sc_core], bf16)
    tot_in = nc.dram_tensor("tot_in", [128, NCORES * 2 * M], f32)
    tot_out = nc.dram_tensor("tot_out", [128, NCORES * 2 * M], f32)

    with tile.TileContext(nc) as tc:
        with (
            tc.tile_pool(name="wts", bufs=2) as wts,
            tc.tile_pool(name="inb", bufs=2) as inb,
            tc.tile_pool(name="act", bufs=2) as actp,
            tc.tile_pool(name="scn", bufs=2) as scn,
            tc.tile_pool(name="tmp", bufs=3) as tmp,
            tc.tile_pool(name="osb", bufs=3) as osbp,
            tc.tile_pool(name="sml", bufs=1) as sml,
            tc.tile_pool(name="ps", bufs=4, space="PSUM") as ps,
            tc.tile_pool(name="pso", bufs=2, space="PSUM") as pso,
        ):
            # weights: tags shared pairwise (wa: wk then wq, wb: wv then
            # wp) with bufs=2 so the phase-B sets can load early.
            def load_w(ext, tagpfx):
                ts_ = []
                for kk in range(K):
                    t = wts.tile([128, H], bf16, tag=f"{tagpfx}{kk}")
                    nc.sync.dma_start(t[:], ext[kk * 128:(kk + 1) * 128, :])
                    ts_.append(t)
                return ts_

            def load_in(ext, tagpfx, c):
                ts_ = []
                for kk in range(K):
                    t = inb.tile([128, SC], bf16, tag=f"{tagpfx}{kk}")
                    nc.sync.dma_start(
                        t[:], ext[kk * 128:(kk + 1) * 128, bass.ts(c, SC)])
                    ts_.append(t)
                return ts_

            def proj(w_t, ins_c, m):
                psm = ps.tile([128, SC], f32, tag="ps")
                for kk in range(K):
                    nc.tensor.matmul(
                        psm[:], w_t[kk][:, m * 128:(m + 1) * 128],
                        ins_c[kk][:], start=(kk == 0), stop=(kk == K - 1))
                return psm

            # Load order matters for the head: the first matmul group
            # needs wk + the first k chunk, so those DMAs go first.
            wk_t = load_w(wk_e, "wa")
            kc0 = load_in(kT_e, "ik", 0)
            vc0 = load_in(vT_e, "iv", 0)
            wv_t = load_w(wv_e, "wb")
            sself = sml.tile([128, NCORES * 2 * M], f32, tag="sself")
            spart = sml.tile([128, NCORES * 2 * M], f32, tag="spart")
            nc.sync.dma_start(sself[:], sself_e[:])
            nc.sync.dma_start(spart[:], spart_e[:])

            # ---- phase A: k/v projections, exp, kv-mult, scans, spills
            wq_t = []
            wp_t = []
            sk_prev = [None] * M
            skv_prev = [None] * M
            for s in range(ns):
                ssl = bass.ts(s, SC)
                if s == 0:
                    kc, vc = kc0, vc0
                else:
                    kc = load_in(kT_e, "ik", s)
                    vc = load_in(vT_e, "iv", s)
                ke = []
                for m in range(M):
                    psm = proj(wk_t, kc, m)
                    t = actp.tile([128, SC], bf16, tag=f"ke{m}")
                    nc.scalar.activation(t[:], psm[:], AF.Exp)
                    ke.append(t)
                for m in range(M):
                    psm = proj(wv_t, vc, m)
                    kv = actp.tile([128, SC], bf16, tag=f"kv{m}")
                    nc.vector.tensor_mul(kv[:], ke[m][:], psm[:])
                    skt = scn.tile([128, SC], bf16, tag=f"sk{m}")
                    init = 0.0 if s == 0 else sk_prev[m][:, SC - 1:SC]
                    nc.vector.tensor_tensor_scan(
                        skt[:], ke[m][:], ke[m][:], init, OP.add, OP.bypass)
                    sk_prev[m] = skt
                    nc.sync.dma_start(
                        sk_sp[m * 128:(m + 1) * 128, ssl], skt[:])
                    skvt = scn.tile([128, SC], bf16, tag=f"sv{m}")
                    init = 0.0 if s == 0 else skv_prev[m][:, SC - 1:SC]
                    nc.vector.tensor_tensor_scan(
                        skvt[:], kv[:], kv[:], init, OP.add, OP.bypass)
                    skv_prev[m] = skvt
                    nc.sync.dma_start(
                        skv_sp[m * 128:(m + 1) * 128, ssl], skvt[:])
                # phase-B weights: a few k-tiles per chunk, spread so
                # the DMA queues never see a 4 MiB weight burst that
                # would delay the next chunk's input loads.
                for kk in range(s * K // ns, (s + 1) * K // ns):
                    t = wts.tile([128, H], bf16, tag=f"wa{kk}",
                                 name=f"wq{kk}")
                    nc.sync.dma_start(t[:], wq_e[kk * 128:(kk + 1) * 128, :])
                    wq_t.append(t)
                    t = wts.tile([128, H], bf16, tag=f"wb{kk}",
                                 name=f"wp{kk}")
                    nc.sync.dma_start(t[:], wp_e[kk * 128:(kk + 1) * 128, :])
                    wp_t.append(t)

            # ---- carry exchange: totals -> slots -> allreduce -> carry
            NSLOT = 2 * M
            tot = sml.tile([128, NSLOT], f32, tag="tot")
            for m in range(M):
                nc.vector.tensor_copy(tot[:, 2 * m:2 * m + 1],
                                      sk_prev[m][:, SC - 1:SC])
                nc.vector.tensor_copy(tot[:, 2 * m + 1:2 * m + 2],
                                      skv_prev[m][:, SC - 1:SC])
            slots = sml.tile([128, NCORES * NSLOT], f32, tag="slots")
            for c in range(NCORES):
                nc.vector.tensor_copy(
                    slots[:, c * NSLOT:(c + 1) * NSLOT], tot[:])
            nc.vector.tensor_mul(slots[:], slots[:], sself[:])
            nc.sync.dma_start(tot_in[:], slots[:])
            nc.gpsimd.collective_compute(
                "AllReduce", OP.add,
                replica_groups=[list(range(NCORES))],
                ins=[tot_in.ap().opt()],
                outs=[tot_out.ap().opt()],
            )
            alltot = sml.tile([128, NCORES * NSLOT], f32, tag="alltot")
            nc.sync.dma_start(alltot[:], tot_out[:])
            nc.vector.tensor_mul(alltot[:], alltot[:], spart[:])
            r4 = sml.tile([128, 4 * NSLOT], f32, tag="r4")
            nc.vector.tensor_add(r4[:], alltot[:, 0:4 * NSLOT],
                                 alltot[:, 4 * NSLOT:8 * NSLOT])
            r2 = sml.tile([128, 2 * NSLOT], f32, tag="r2")
            nc.vector.tensor_add(r2[:], r4[:, 0:2 * NSLOT],
                                 r4[:, 2 * NSLOT:4 * NSLOT])
            carry = sml.tile([128, NSLOT], f32, tag="carry")
            nc.vector.tensor_add(carry[:], r2[:, 0:NSLOT],
                                 r2[:, NSLOT:2 * NSLOT])

            # ---- phase B: EQ + sigmoid, final elementwise, out
            #      projection.  Emission skewed one chunk: EQ(s+1) is
            #      emitted before OUT(s) so the PE stream rides through
            #      the collective latency.
            def emit_eq(s):
                ssl = bass.ts(s, SC)
                qc = load_in(qT_e, "ik", s)
                sg = []
                lsk = []
                lskv = []
                for m in range(M):
                    psm = proj(wq_t, qc, m)
                    t = actp.tile([128, SC], bf16, tag=f"ke{m}")
                    nc.scalar.activation(t[:], psm[:], AF.Sigmoid)
                    sg.append(t)
                for m in range(M):
                    t = actp.tile([128, SC], bf16, tag=f"kv{m}")
                    nc.sync.dma_start(t[:], sk_sp[m * 128:(m + 1) * 128, ssl])
                    lsk.append(t)
                    t = scn.tile([128, SC], bf16, tag=f"sk{m}")
                    nc.sync.dma_start(t[:],
                                      skv_sp[m * 128:(m + 1) * 128, ssl])
                    lskv.append(t)
                return sg, lsk, lskv

            def emit_out(s, state):
                sg, lsk, lskv = state
                ys = []
                for m in range(M):
                    den = tmp.tile([128, SC], f32, tag="den")
                    nc.scalar.activation(den[:], lsk[m][:], AF.Identity,
                                         bias=carry[:, 2 * m:2 * m + 1])
                    num = tmp.tile([128, SC], f32, tag="num")
                    nc.scalar.activation(num[:], lskv[m][:], AF.Identity,
                                         bias=carry[:, 2 * m + 1:2 * m + 2])
                    rcp = tmp.tile([128, SC], f32, tag="num")
                    nc.vector.reciprocal_approx_fast(rcp[:], den[:])
                    rat = tmp.tile([128, SC], f32, tag="den")
                    nc.vector.tensor_mul(rat[:], num[:], rcp[:])
                    y = scn.tile([128, SC], bf16, tag=f"sv{m}")
                    nc.vector.tensor_mul(y[:], rat[:], sg[m][:])
                    ys.append(y)
                for t4 in range(NT):
                    psm = pso.tile([128, 1024], f32)
                    tsl = bass.ts(t4, 128)
                    for m in range(M):
                        for n in range(2):
                            nc.tensor.matmul(
                                psm[:, n * 512:(n + 1) * 512],
                                ys[m][:, tsl],
                                wp_t[m][:, n * 512:(n + 1) * 512],
                                start=(m == 0), stop=(m == M - 1))
                    ob = osbp.tile([128, 1024], f32, tag="ob")
                    nc.scalar.copy(ob[:], psm[:])
                    nc.sync.dma_start(
                        out_e[s * SC + t4 * 128:s * SC + (t4 + 1) * 128, :],
                        ob[:])

            # skew-2: two EQ chunks of PE work queued ahead of each OUT
            # chunk cover most of the collective's ~40 us latency.
            skew = min(2, ns)
            states = {}
            for s in range(skew):
                states[s] = emit_eq(s)
            for s in range(skew, ns):
                emit_out(s - skew, states.pop(s - skew))
                states[s] = emit_eq(s)
            for s in range(ns - skew, ns):
                emit_out(s, states.pop(s))

    nc.compile()
    prune_pe_incs(nc, verbose=True)
    return nc


def _in_maps(q, k, v, w_q, w_k, w_v, w_p):
    bf = ml_dtypes.bfloat16
    ws = {n: np.ascontiguousarray(w, dtype=bf)
          for n, w in (("wq", w_q), ("wk", w_k), ("wv", w_v), ("wp", w_p))}
    NSLOT = 2 * M
    in_maps = []
    for c in range(NCORES):
        b, h = c // 2, c % 2
        sl = slice(h * SC_CORE, (h + 1) * SC_CORE)
        sself = np.zeros((128, NCORES * NSLOT), np.float32)
        sself[:, c * NSLOT:(c + 1) * NSLOT] = 1.0
        spart = np.zeros((128, NCORES * NSLOT), np.float32)
        if h == 1:
            p = c ^ 1
            spart[:, p * NSLOT:(p + 1) * NSLOT] = 1.0
        in_maps.append({
            "qT": np.ascontiguousarray(q[b, sl].T, dtype=bf),
            "kT": np.ascontiguousarray(k[b, sl].T, dtype=bf),
            "vT": np.ascontiguousarray(v[b, sl].T, dtype=bf),
            **ws,
            "slot_self": sself,
            "slot_partner": spart,
        })
    return in_maps


def run(q, k, v, w_q, w_k, w_v, w_p, trace=False, tmpdir=None):
    if "nc" not in _cache:
        _cache["nc"] = build()
    nc = _cache["nc"]
    in_maps = _in_maps(q, k, v, w_q, w_k, w_v, w_p)
    res = run_bass_kernel_spmd(nc, in_maps, core_ids=list(range(NCORES)),
                               trace=trace, tmpdir=tmpdir)
    out = np.empty((B, S, H), np.float32)
    for c in range(NCORES):
        b, h = c // 2, c % 2
        out[b, h * SC_CORE:(h + 1) * SC_CORE, :] = res.results[c]["out"]
    return out, res


def kernel(**inputs):
    out, _ = run(**{k: np.asarray(v) for k, v in inputs.items()})
    return out



# revision 5
# speedup vs baseline: 1.4391x; 1.4391x over previous
"""AFT-simple attention (nn_AsfAttention) on 8 TRN2 NeuronCores.

Reference (per batch b):
    emb_q = q @ w_q; emb_k = k @ w_k; emb_v = v @ w_v
    k_exp = exp(emb_k)
    y = sigmoid(emb_q) * cumsum(k_exp * emb_v, seq) / cumsum(k_exp, seq)
    out = y @ w_p

Sharding (v2): core c = 2*b + g handles batch b and OUT-CHANNEL half g
(512 of 1024 channels) over the FULL 8192 sequence.  The per-channel
cumsum is then fully core-local: no carry exchange, no collective, no
mid-kernel stall.  The price is that the final projection out = y @ w_p
only has half the contraction rows per core, so each core emits a
partial [S, H] output and the host sums the two partials per batch
(cheap numpy add during unshard).

Precision: q and k projections run fp8-e4m3 with DoubleRow perf mode
(2 contraction rows per PE cell -> ~1.8x matmul throughput); their
error is gated by sigmoid (q) and damped by the cumsum ratio (k),
measured 1.3e-2 rel total vs the 2e-2 budget.  v and p projections
stay bf16 (fp8 there costs ~4e-2).  Weights are pre-scaled by 128 for
fp8 (w ~ 0.01 would be subnormal in e4m3); the 1/128 unscale rides the
exp/sigmoid activation's scale input for free.

Layout: channel-major on chip ([ch partitions, seq free]) so the
projections use weights as the stationary operand, the seq cumsum is a
native tensor_tensor_scan along the free dim, and the output
projection takes y tiles as lhsT yielding natural [seq, ch_out] rows.
Everything streams in one phase: per 512-seq chunk do k/v/q
projections + scans + elementwise + out projection, with the out
projection software-pipelined one chunk behind the projections.
"""

import numpy as np
import ml_dtypes

import concourse.bass as bass
import concourse.tile as tile
from concourse import bacc, mybir
from concourse.bass_utils import run_bass_kernel_spmd

B, S, H = 4, 8192, 1024
NCORES = 8
GH = 512                  # out-channel half per core
SC = 512                  # seq chunk (columns per matmul / scan step)
NCH = S // SC             # 16 seq chunks
MO = GH // 128            # 4 out-channel tiles per core
KI = H // 128             # 8 contraction subtiles (bf16 path)
KI2 = H // 256            # 4 fp8 DoubleRow contraction pairs
NT = SC // 128            # 4 seq subtiles per chunk for the out projection
WSCALE = 128.0            # fp8 weight pre-scale

bf16 = mybir.dt.bfloat16
f8 = mybir.dt.float8e4
f32 = mybir.dt.float32
AF = mybir.ActivationFunctionType
OP = mybir.AluOpType
DR = mybir.MatmulPerfMode.DoubleRow

_cache = {}


def prune_pe_incs(nc, verbose=False):
    """Drop the per-matmul PE semaphore increment from every matmul that
    is not the end of its accumulation group (stop_tensor_calc), then
    renumber all waits on that semaphore. A wait whose original target
    inc was dropped is rounded UP to the next kept inc — safe here
    because kept-inc matmuls never depend on the rounded-up waiters
    (they only consume earlier-generation tiles).

    Each inc is a serialized EVT_SEM register write (~26 ns) on the PE
    sequencer; at ~1500 matmuls this is ~40 us of PE issue overhead.
    """
    import bisect
    from collections import defaultdict

    insts = []
    for bb in nc.main_func.blocks:
        insts.extend(bb.instructions)

    upd = defaultdict(list)
    for pos, ins in enumerate(insts):
        si = ins.sync_info
        if not si:
            continue
        for u in si.on_update:
            upd[u.ant_name].append((pos, ins, u))

    changed = 0
    for sem, us in upd.items():
        if len(us) < 64:
            continue
        if not all(type(i).__name__ == "InstMatmult"
                   and u.update_mode == "sem-inc" and u.update_value == 1
                   for _, i, u in us):
            continue
        kept = [bool(i.stop_tensor_calc) for _, i, _ in us]
        kept[-1] = True
        kept_idx = [j for j, k in enumerate(kept) if k]

        def new_thresh(t):
            j = bisect.bisect_left(kept_idx, t - 1)
            assert j < len(kept_idx), f"wait {t} beyond last kept inc"
            return j + 1

        for ins in insts:
            si = ins.sync_info
            if not si:
                continue
            for w in si.on_wait:
                if w.ant_name == sem:
                    assert w.wait_mode == "sem-ge-imm", w.wait_mode
                    w.wait_value = new_thresh(w.wait_value)
        for j, (_, ins, u) in enumerate(us):
            if not kept[j]:
                si = ins.sync_info
                rest = [x for x in si.on_update if x.ant_name != sem]
                assert len(rest) == len(si.on_update) - 1
                if rest:
                    si.on_update[:] = rest
                else:
                    si.on_update.clear()
                changed += 1
    if verbose:
        print(f"prune_pe_incs: removed {changed} matmul sem-incs")
    return changed


def build(debug=False):
    nc = bacc.Bacc("TRN2", target_bir_lowering=False, debug=debug,
                   num_devices=1)

    qT_e = nc.dram_tensor("qT8", [H, S], f8, kind="ExternalInput")
    kT_e = nc.dram_tensor("kT8", [H, S], f8, kind="ExternalInput")
    vT_e = nc.dram_tensor("vT", [H, S], bf16, kind="ExternalInput")
    wq_e = nc.dram_tensor("wq8", [H, GH], f8, kind="ExternalInput")
    wk_e = nc.dram_tensor("wk8", [H, GH], f8, kind="ExternalInput")
    wv_e = nc.dram_tensor("wvb", [H, GH], bf16, kind="ExternalInput")
    wp_e = nc.dram_tensor("wpb", [GH, H], bf16, kind="ExternalInput")
    out_e = nc.dram_tensor("out", [S, H], f32, kind="ExternalOutput")

    with tile.TileContext(nc) as tc:
        with (
            tc.tile_pool(name="wts", bufs=1) as wts,
            tc.tile_pool(name="inb", bufs=2) as inb,
            tc.tile_pool(name="act", bufs=2) as actp,
            tc.tile_pool(name="scn", bufs=2) as scn,
            tc.tile_pool(name="sgp", bufs=2) as sgp,
            tc.tile_pool(name="yp", bufs=2) as yp,
            tc.tile_pool(name="tmp", bufs=3) as tmp,
            tc.tile_pool(name="osb", bufs=3) as osbp,
            tc.tile_pool(name="ps", bufs=3, space="PSUM") as ps,
            tc.tile_pool(name="pso", bufs=2, space="PSUM") as pso,
        ):
            # ---- weights (resident all kernel) -----------------------
            # Load order = first-use order: wk + first k chunk head the
            # DMA queues so the PE can start within a few us.
            def load_w(ext, name, dtype):
                t = wts.tile([128, KI if ext is not wp_e else MO,
                              GH if ext is not wp_e else H], dtype, tag=name)
                nsub = KI if ext is not wp_e else MO
                for kk in range(nsub):
                    nc.sync.dma_start(
                        t[:, kk, :], ext[kk * 128:(kk + 1) * 128, :])
                return t

            wk_t = load_w(wk_e, "wk", f8)

            def load_in(ext, tagpfx, dtype, s):
                t = inb.tile([128, KI, SC], dtype, tag=tagpfx)
                for kk in range(KI):
                    nc.sync.dma_start(
                        t[:, kk, :],
                        ext[kk * 128:(kk + 1) * 128, bass.ts(s, SC)])
                return t

            kc0 = load_in(kT_e, "ik", f8, 0)
            wv_t = load_w(wv_e, "wv", bf16)
            vc0 = load_in(vT_e, "iv", bf16, 0)
            wq_t = load_w(wq_e, "wq", f8)
            qc0 = load_in(qT_e, "iq", f8, 0)
            wp_t = load_w(wp_e, "wp", bf16)

            sk_prev = [None] * MO
            skv_prev = [None] * MO

            def emit_proj(s, pre=None):
                if pre is None:
                    kc = load_in(kT_e, "ik", f8, s)
                    vc = load_in(vT_e, "iv", bf16, s)
                    qc = load_in(qT_e, "iq", f8, s)
                else:
                    kc, vc, qc = pre
                # k projection (fp8 DoubleRow) + exp
                ke = []
                for m in range(MO):
                    psm = ps.tile([128, SC], f32, tag="ps")
                    for kk in range(KI2):
                        nc.tensor.matmul(
                            psm[:],
                            wk_t[:, 2 * kk:2 * kk + 2, m * 128:(m + 1) * 128],
                            kc[:, 2 * kk:2 * kk + 2, :],
                            start=(kk == 0), stop=(kk == KI2 - 1),
                            perf_mode=DR)
                    t = actp.tile([128, SC], bf16, tag=f"ke{m}")
                    nc.scalar.activation(t[:], psm[:], AF.Exp,
                                         scale=1.0 / WSCALE)
                    ke.append(t)
                # v projection (bf16) + kv mult + both scans
                for m in range(MO):
                    psm = ps.tile([128, SC], f32, tag="ps")
                    for kk in range(KI):
                        nc.tensor.matmul(
                            psm[:],
                            wv_t[:, kk, m * 128:(m + 1) * 128],
                            vc[:, kk, :],
                            start=(kk == 0), stop=(kk == KI - 1))
                    kv = actp.tile([128, SC], bf16, tag=f"kv{m}")
                    nc.vector.tensor_mul(kv[:], ke[m][:], psm[:])
                    skt = scn.tile([128, SC], f32, tag=f"sk{m}")
                    init = 0.0 if s == 0 else sk_prev[m][:, SC - 1:SC]
                    nc.vector.tensor_tensor_scan(
                        skt[:], ke[m][:], ke[m][:], init, OP.add, OP.bypass)
                    sk_prev[m] = skt
                    skvt = scn.tile([128, SC], bf16, tag=f"sv{m}")
                    init = 0.0 if s == 0 else skv_prev[m][:, SC - 1:SC]
                    nc.vector.tensor_tensor_scan(
                        skvt[:], kv[:], kv[:], init, OP.add, OP.bypass)
                    skv_prev[m] = skvt
                # q projection (fp8 DoubleRow) + sigmoid
                sg = []
                for m in range(MO):
                    psm = ps.tile([128, SC], f32, tag="ps")
                    for kk in range(KI2):
                        nc.tensor.matmul(
                            psm[:],
                            wq_t[:, 2 * kk:2 * kk + 2, m * 128:(m + 1) * 128],
                            qc[:, 2 * kk:2 * kk + 2, :],
                            start=(kk == 0), stop=(kk == KI2 - 1),
                            perf_mode=DR)
                    t = sgp.tile([128, SC], bf16, tag=f"sg{m}")
                    nc.scalar.activation(t[:], psm[:], AF.Sigmoid,
                                         scale=1.0 / WSCALE)
                    sg.append(t)
                return sg, [sk_prev[m] for m in range(MO)], \
                    [skv_prev[m] for m in range(MO)]

            def emit_out(s, state):
                sg, sks, skvs = state
                ys = []
                for m in range(MO):
                    rcp = tmp.tile([128, SC], f32, tag="rcp")
                    nc.vector.reciprocal_approx_fast(rcp[:], sks[m][:])
                    rat = tmp.tile([128, SC], bf16, tag="rat")
                    nc.vector.tensor_mul(rat[:], skvs[m][:], rcp[:])
                    y = yp.tile([128, SC], bf16, tag=f"y{m}")
                    nc.vector.tensor_mul(y[:], rat[:], sg[m][:])
                    ys.append(y)
                for t4 in range(NT):
                    psm = pso.tile([128, H], f32)
                    tsl = bass.ts(t4, 128)
                    for m in range(MO):
                        for n in range(2):
                            nc.tensor.matmul(
                                psm[:, n * 512:(n + 1) * 512],
                                ys[m][:, tsl],
                                wp_t[:, m, n * 512:(n + 1) * 512],
                                start=(m == 0), stop=(m == MO - 1))
                    ob = osbp.tile([128, H], f32, tag="ob")
                    nc.scalar.copy(ob[:], psm[:])
                    nc.sync.dma_start(
                        out_e[s * SC + t4 * 128:s * SC + (t4 + 1) * 128, :],
                        ob[:])

            # software pipeline: out projection rides one chunk behind
            states = {0: emit_proj(0, pre=(kc0, vc0, qc0))}
            for s in range(1, NCH):
                states[s] = emit_proj(s)
                emit_out(s - 1, states.pop(s - 1))
            emit_out(NCH - 1, states.pop(NCH - 1))

    nc.compile()
    prune_pe_incs(nc, verbose=True)
    return nc


def _in_maps(q, k, v, w_q, w_k, w_v, w_p):
    bf = ml_dtypes.bfloat16
    e4 = ml_dtypes.float8_e4m3
    per_b = []
    for b in range(B):
        per_b.append({
            "qT8": np.ascontiguousarray(q[b].T).astype(e4),
            "kT8": np.ascontiguousarray(k[b].T).astype(e4),
            "vT": np.ascontiguousarray(v[b].T).astype(bf),
        })
    per_g = []
    for g in range(2):
        sl = slice(g * GH, (g + 1) * GH)
        per_g.append({
            "wq8": (w_q[:, sl] * WSCALE).astype(e4),
            "wk8": (w_k[:, sl] * WSCALE).astype(e4),
            "wvb": w_v[:, sl].astype(bf),
            "wpb": w_p[sl, :].astype(bf),
        })
    return [{**per_b[c // 2], **per_g[c % 2]} for c in range(NCORES)]


def run(q, k, v, w_q, w_k, w_v, w_p, trace=False, tmpdir=None):
    if "nc" not in _cache:
        _cache["nc"] = build()
    nc = _cache["nc"]
    in_maps = _in_maps(q, k, v, w_q, w_k, w_v, w_p)
    res = run_bass_kernel_spmd(nc, in_maps, core_ids=list(range(NCORES)),
                               trace=trace, tmpdir=tmpdir)
    out = np.empty((B, S, H), np.float32)
    for b in range(B):
        out[b] = res.results[2 * b]["out"]
        out[b] += res.results[2 * b + 1]["out"]
    return out, res


def kernel(**inputs):
    out, _ = run(**{k: np.asarray(v) for k, v in inputs.items()})
    return out


# revision 8
# speedup vs baseline: 1.6477x; 1.1449x over previous
"""AFT-simple attention (nn_AsfAttention) on 8 TRN2 NeuronCores.

Reference (per batch b):
    emb_q = q @ w_q; emb_k = k @ w_k; emb_v = v @ w_v
    k_exp = exp(emb_k)
    y = sigmoid(emb_q) * cumsum(k_exp * emb_v, seq) / cumsum(k_exp, seq)
    out = y @ w_p

Sharding (v2): core c = 2*b + g handles batch b and OUT-CHANNEL half g
(512 of 1024 channels) over the FULL 8192 sequence.  The per-channel
cumsum is then fully core-local: no carry exchange, no collective, no
mid-kernel stall.  The price is that the final projection out = y @ w_p
only has half the contraction rows per core, so each core emits a
partial [S, H] output and the host sums the two partials per batch
(cheap numpy add during unshard).

Precision: q and k projections run fp8-e4m3 with DoubleRow perf mode
(2 contraction rows per PE cell -> ~1.8x matmul throughput); their
error is gated by sigmoid (q) and damped by the cumsum ratio (k),
measured 1.3e-2 rel total vs the 2e-2 budget.  v and p projections
stay bf16 (fp8 there costs ~4e-2).  Weights are pre-scaled by 128 for
fp8 (w ~ 0.01 would be subnormal in e4m3); the 1/128 unscale rides the
exp/sigmoid activation's scale input for free.

Layout: channel-major on chip ([ch partitions, seq free]) so the
projections use weights as the stationary operand, the seq cumsum is a
native tensor_tensor_scan along the free dim, and the output
projection takes y tiles as lhsT yielding natural [seq, ch_out] rows.
Everything streams in one phase: per 512-seq chunk do k/v/q
projections + scans + elementwise + out projection, with the out
projection software-pipelined one chunk behind the projections.
"""

import numpy as np
import ml_dtypes

import concourse.bass as bass
import concourse.tile as tile
from concourse import bacc, mybir
from concourse.bass_utils import run_bass_kernel_spmd

B, S, H = 4, 8192, 1024
NCORES = 8
GH = 512                  # out-channel half per core
SC = 512                  # seq chunk (columns per matmul / scan step)
NCH = S // SC             # 16 seq chunks
MO = GH // 128            # 4 out-channel tiles per core
KI = H // 128             # 8 contraction subtiles (bf16 path)
KI2 = H // 256            # 4 fp8 DoubleRow contraction pairs
NT = SC // 128            # 4 seq subtiles per chunk for the out projection
WSCALE = 128.0            # fp8 weight pre-scale

bf16 = mybir.dt.bfloat16
f8 = mybir.dt.float8e4
f32 = mybir.dt.float32
AF = mybir.ActivationFunctionType
OP = mybir.AluOpType
DR = mybir.MatmulPerfMode.DoubleRow

_cache = {}


def prune_pe_incs(nc, verbose=False):
    """Drop the per-matmul PE semaphore increment from every matmul that
    is not the end of its accumulation group (stop_tensor_calc), then
    renumber all waits on that semaphore. A wait whose original target
    inc was dropped is rounded UP to the next kept inc — safe here
    because kept-inc matmuls never depend on the rounded-up waiters
    (they only consume earlier-generation tiles).

    Each inc is a serialized EVT_SEM register write (~26 ns) on the PE
    sequencer; at ~1500 matmuls this is ~40 us of PE issue overhead.
    """
    import bisect
    from collections import defaultdict

    insts = []
    for bb in nc.main_func.blocks:
        insts.extend(bb.instructions)

    upd = defaultdict(list)
    for pos, ins in enumerate(insts):
        si = ins.sync_info
        if not si:
            continue
        for u in si.on_update:
            upd[u.ant_name].append((pos, ins, u))

    changed = 0
    for sem, us in upd.items():
        if len(us) < 64:
            continue
        if not all(type(i).__name__ == "InstMatmult"
                   and u.update_mode == "sem-inc" and u.update_value == 1
                   for _, i, u in us):
            continue
        kept = [bool(i.stop_tensor_calc) for _, i, _ in us]
        kept[-1] = True
        kept_idx = [j for j, k in enumerate(kept) if k]

        def new_thresh(t):
            j = bisect.bisect_left(kept_idx, t - 1)
            assert j < len(kept_idx), f"wait {t} beyond last kept inc"
            return j + 1

        for ins in insts:
            si = ins.sync_info
            if not si:
                continue
            for w in si.on_wait:
                if w.ant_name == sem:
                    assert w.wait_mode == "sem-ge-imm", w.wait_mode
                    w.wait_value = new_thresh(w.wait_value)
        for j, (_, ins, u) in enumerate(us):
            if not kept[j]:
                si = ins.sync_info
                rest = [x for x in si.on_update if x.ant_name != sem]
                assert len(rest) == len(si.on_update) - 1
                if rest:
                    si.on_update[:] = rest
                else:
                    si.on_update.clear()
                changed += 1
    if verbose:
        print(f"prune_pe_incs: removed {changed} matmul sem-incs")
    return changed


def build(debug=False):
    nc = bacc.Bacc("TRN2", target_bir_lowering=False, debug=debug,
                   num_devices=1)

    qT_e = nc.dram_tensor("qT8", [H, S], f8, kind="ExternalInput")
    kT_e = nc.dram_tensor("kT8", [H, S], f8, kind="ExternalInput")
    vT_e = nc.dram_tensor("vT", [H, S], bf16, kind="ExternalInput")
    wq_e = nc.dram_tensor("wq8", [H, GH], f8, kind="ExternalInput")
    wk_e = nc.dram_tensor("wk8", [H, GH], f8, kind="ExternalInput")
    wv_e = nc.dram_tensor("wvb", [H, GH], bf16, kind="ExternalInput")
    wp_e = nc.dram_tensor("wpb", [GH, H], bf16, kind="ExternalInput")
    out_e = nc.dram_tensor("out", [S, H], f32, kind="ExternalOutput")

    with tile.TileContext(nc) as tc:
        with (
            tc.tile_pool(name="wts", bufs=1) as wts,
            tc.tile_pool(name="inb", bufs=2) as inb,
            tc.tile_pool(name="act", bufs=2) as actp,
            tc.tile_pool(name="scn", bufs=2) as scn,
            tc.tile_pool(name="sgp", bufs=2) as sgp,
            tc.tile_pool(name="yp", bufs=2) as yp,
            tc.tile_pool(name="tmp", bufs=3) as tmp,
            tc.tile_pool(name="osb", bufs=3) as osbp,
            tc.tile_pool(name="ps", bufs=3, space="PSUM") as ps,
            tc.tile_pool(name="pso", bufs=2, space="PSUM") as pso,
        ):
            # ---- weights (resident all kernel) -----------------------
            # Load order = first-use order: wk + first k chunk head the
            # DMA queues so the PE can start within a few us.
            def load_w(ext, name, dtype):
                t = wts.tile([128, KI if ext is not wp_e else MO,
                              GH if ext is not wp_e else H], dtype, tag=name)
                nsub = KI if ext is not wp_e else MO
                for kk in range(nsub):
                    nc.sync.dma_start(
                        t[:, kk, :], ext[kk * 128:(kk + 1) * 128, :])
                return t

            wk_t = load_w(wk_e, "wk", f8)

            def load_in(ext, tagpfx, dtype, s):
                t = inb.tile([128, KI, SC], dtype, tag=tagpfx)
                for kk in range(KI):
                    nc.sync.dma_start(
                        t[:, kk, :],
                        ext[kk * 128:(kk + 1) * 128, bass.ts(s, SC)])
                return t

            kc0 = load_in(kT_e, "ik", f8, 0)
            wv_t = load_w(wv_e, "wv", bf16)
            vc0 = load_in(vT_e, "iv", bf16, 0)
            wq_t = load_w(wq_e, "wq", f8)
            qc0 = load_in(qT_e, "iq", f8, 0)
            wp_t = load_w(wp_e, "wp", bf16)

            sk_prev = [None] * MO
            skv_prev = [None] * MO

            def emit_proj(s, pre=None):
                if pre is None:
                    kc = load_in(kT_e, "ik", f8, s)
                    vc = load_in(vT_e, "iv", bf16, s)
                    qc = load_in(qT_e, "iq", f8, s)
                else:
                    kc, vc, qc = pre
                # k projection (fp8 DoubleRow) + exp
                ke = []
                for m in range(MO):
                    psm = ps.tile([128, SC], f32, tag="ps")
                    for kk in range(KI2):
                        nc.tensor.matmul(
                            psm[:],
                            wk_t[:, 2 * kk:2 * kk + 2, m * 128:(m + 1) * 128],
                            kc[:, 2 * kk:2 * kk + 2, :],
                            start=(kk == 0), stop=(kk == KI2 - 1),
                            perf_mode=DR)
                    t = actp.tile([128, SC], bf16, tag=f"ke{m}")
                    nc.scalar.activation(t[:], psm[:], AF.Exp,
                                         scale=1.0 / WSCALE)
                    ke.append(t)
                # v projection (bf16) + kv mult + both scans
                for m in range(MO):
                    psm = ps.tile([128, SC], f32, tag="ps")
                    for kk in range(KI):
                        nc.tensor.matmul(
                            psm[:],
                            wv_t[:, kk, m * 128:(m + 1) * 128],
                            vc[:, kk, :],
                            start=(kk == 0), stop=(kk == KI - 1))
                    skt = scn.tile([128, SC], f32, tag=f"sk{m}")
                    init = 0.0 if s == 0 else sk_prev[m][:, SC - 1:SC]
                    nc.vector.tensor_tensor_scan(
                        skt[:], ke[m][:], ke[m][:], init, OP.add, OP.bypass)
                    sk_prev[m] = skt
                    kv = actp.tile([128, SC], bf16, tag=f"kv{m}")
                    nc.vector.tensor_mul(kv[:], ke[m][:], psm[:])
                    skvt = scn.tile([128, SC], bf16, tag=f"sv{m}")
                    init = 0.0 if s == 0 else skv_prev[m][:, SC - 1:SC]
                    nc.vector.tensor_tensor_scan(
                        skvt[:], kv[:], kv[:], init, OP.add, OP.bypass)
                    skv_prev[m] = skvt
                # q projection (fp8 DoubleRow) + sigmoid
                sg = []
                for m in range(MO):
                    psm = ps.tile([128, SC], f32, tag="ps")
                    for kk in range(KI2):
                        nc.tensor.matmul(
                            psm[:],
                            wq_t[:, 2 * kk:2 * kk + 2, m * 128:(m + 1) * 128],
                            qc[:, 2 * kk:2 * kk + 2, :],
                            start=(kk == 0), stop=(kk == KI2 - 1),
                            perf_mode=DR)
                    t = sgp.tile([128, SC], bf16, tag=f"sg{m}")
                    nc.scalar.activation(t[:], psm[:], AF.Sigmoid,
                                         scale=1.0 / WSCALE)
                    sg.append(t)
                return sg, [sk_prev[m] for m in range(MO)], \
                    [skv_prev[m] for m in range(MO)]

            def emit_ydve(state):
                """DVE-only part of the out phase: y = sg * skv / sk.
                Emitted BEFORE the next chunk's scans so y lands in the
                DVE FIFO ahead of them — the out projection then never
                waits on a y that is queued behind 10us of scans."""
                sg, sks, skvs = state
                ys = []
                for m in range(MO):
                    rcp = tmp.tile([128, SC], f32, tag="rcp")
                    nc.vector.reciprocal_approx_fast(rcp[:], sks[m][:])
                    rat = tmp.tile([128, SC], bf16, tag="rat")
                    nc.vector.tensor_mul(rat[:], skvs[m][:], rcp[:])
                    y = yp.tile([128, SC], bf16, tag=f"y{m}")
                    nc.vector.tensor_mul(y[:], rat[:], sg[m][:])
                    ys.append(y)
                return ys

            def emit_out(s, ys):
                for t4 in range(NT):
                    psm = pso.tile([128, H], f32)
                    tsl = bass.ts(t4, 128)
                    for m in range(MO):
                        for n in range(2):
                            nc.tensor.matmul(
                                psm[:, n * 512:(n + 1) * 512],
                                ys[m][:, tsl],
                                wp_t[:, m, n * 512:(n + 1) * 512],
                                start=(m == 0), stop=(m == MO - 1))
                    ob = osbp.tile([128, H], f32, tag="ob")
                    nc.scalar.copy(ob[:], psm[:])
                    nc.sync.dma_start(
                        out_e[s * SC + t4 * 128:s * SC + (t4 + 1) * 128, :],
                        ob[:])

            # software pipeline: out projection rides one chunk behind
            states = {0: emit_proj(0, pre=(kc0, vc0, qc0))}
            for s in range(1, NCH):
                ys = emit_ydve(states.pop(s - 1))
                states[s] = emit_proj(s)
                emit_out(s - 1, ys)
            ys = emit_ydve(states.pop(NCH - 1))
            emit_out(NCH - 1, ys)

    nc.compile()
    prune_pe_incs(nc, verbose=True)
    return nc


def _in_maps(q, k, v, w_q, w_k, w_v, w_p):
    bf = ml_dtypes.bfloat16
    e4 = ml_dtypes.float8_e4m3
    per_b = []
    for b in range(B):
        per_b.append({
            "qT8": np.ascontiguousarray(q[b].T).astype(e4),
            "kT8": np.ascontiguousarray(k[b].T).astype(e4),
            "vT": np.ascontiguousarray(v[b].T).astype(bf),
        })
    per_g = []
    for g in range(2):
        sl = slice(g * GH, (g + 1) * GH)
        per_g.append({
            "wq8": (w_q[:, sl] * WSCALE).astype(e4),
            "wk8": (w_k[:, sl] * WSCALE).astype(e4),
            "wvb": w_v[:, sl].astype(bf),
            "wpb": w_p[sl, :].astype(bf),
        })
    return [{**per_b[c // 2], **per_g[c % 2]} for c in range(NCORES)]


def run(q, k, v, w_q, w_k, w_v, w_p, trace=False, tmpdir=None):
    if "nc" not in _cache:
        _cache["nc"] = build()
    nc = _cache["nc"]
    in_maps = _in_maps(q, k, v, w_q, w_k, w_v, w_p)
    res = run_bass_kernel_spmd(nc, in_maps, core_ids=list(range(NCORES)),
                               trace=trace, tmpdir=tmpdir)
    out = np.empty((B, S, H), np.float32)
    for b in range(B):
        out[b] = res.results[2 * b]["out"]
        out[b] += res.results[2 * b + 1]["out"]
    return out, res


def kernel(**inputs):
    out, _ = run(**{k: np.asarray(v) for k, v in inputs.items()})
    return out


# revision 12
# speedup vs baseline: 1.6492x; 1.0009x over previous
"""AFT-simple attention (nn_AsfAttention) on 8 TRN2 NeuronCores.

Reference (per batch b):
    emb_q = q @ w_q; emb_k = k @ w_k; emb_v = v @ w_v
    k_exp = exp(emb_k)
    y = sigmoid(emb_q) * cumsum(k_exp * emb_v, seq) / cumsum(k_exp, seq)
    out = y @ w_p

Sharding (v2): core c = 2*b + g handles batch b and OUT-CHANNEL half g
(512 of 1024 channels) over the FULL 8192 sequence.  The per-channel
cumsum is then fully core-local: no carry exchange, no collective, no
mid-kernel stall.  The price is that the final projection out = y @ w_p
only has half the contraction rows per core, so each core emits a
partial [S, H] output and the host sums the two partials per batch
(cheap numpy add during unshard).

Precision: q and k projections run fp8-e4m3 with DoubleRow perf mode
(2 contraction rows per PE cell -> ~1.8x matmul throughput); their
error is gated by sigmoid (q) and damped by the cumsum ratio (k),
measured 1.3e-2 rel total vs the 2e-2 budget.  v and p projections
stay bf16 (fp8 there costs ~4e-2).  Weights are pre-scaled by 128 for
fp8 (w ~ 0.01 would be subnormal in e4m3); the 1/128 unscale rides the
exp/sigmoid activation's scale input for free.

Layout: channel-major on chip ([ch partitions, seq free]) so the
projections use weights as the stationary operand, the seq cumsum is a
native tensor_tensor_scan along the free dim, and the output
projection takes y tiles as lhsT yielding natural [seq, ch_out] rows.
Everything streams in one phase: per 512-seq chunk do k/v/q
projections + scans + elementwise + out projection, with the out
projection software-pipelined one chunk behind the projections.
"""

import numpy as np
import ml_dtypes

import concourse.bass as bass
import concourse.tile as tile
from concourse import bacc, mybir
from concourse.bass_utils import run_bass_kernel_spmd

B, S, H = 4, 8192, 1024
NCORES = 8
GH = 512                  # out-channel half per core
SC = 512                  # seq chunk (columns per matmul / scan step)
NCH = S // SC             # 16 seq chunks
MO = GH // 128            # 4 out-channel tiles per core
KI = H // 128             # 8 contraction subtiles (bf16 path)
KI2 = H // 256            # 4 fp8 DoubleRow contraction pairs
NT = SC // 128            # 4 seq subtiles per chunk for the out projection
WSCALE = 128.0            # fp8 weight pre-scale

bf16 = mybir.dt.bfloat16
f8 = mybir.dt.float8e4
f32 = mybir.dt.float32
AF = mybir.ActivationFunctionType
OP = mybir.AluOpType
DR = mybir.MatmulPerfMode.DoubleRow

_cache = {}


def prune_pe_incs(nc, verbose=False):
    """Drop the per-matmul PE semaphore increment from every matmul that
    is not the end of its accumulation group (stop_tensor_calc), then
    renumber all waits on that semaphore. A wait whose original target
    inc was dropped is rounded UP to the next kept inc — safe here
    because kept-inc matmuls never depend on the rounded-up waiters
    (they only consume earlier-generation tiles).

    Each inc is a serialized EVT_SEM register write (~26 ns) on the PE
    sequencer; at ~1500 matmuls this is ~40 us of PE issue overhead.
    """
    import bisect
    from collections import defaultdict

    insts = []
    for bb in nc.main_func.blocks:
        insts.extend(bb.instructions)

    upd = defaultdict(list)
    for pos, ins in enumerate(insts):
        si = ins.sync_info
        if not si:
            continue
        for u in si.on_update:
            upd[u.ant_name].append((pos, ins, u))

    changed = 0
    for sem, us in upd.items():
        if len(us) < 64:
            continue
        if not all(type(i).__name__ == "InstMatmult"
                   and u.update_mode == "sem-inc" and u.update_value == 1
                   for _, i, u in us):
            continue
        kept = [bool(i.stop_tensor_calc) for _, i, _ in us]
        kept[-1] = True
        kept_idx = [j for j, k in enumerate(kept) if k]

        def new_thresh(t):
            j = bisect.bisect_left(kept_idx, t - 1)
            assert j < len(kept_idx), f"wait {t} beyond last kept inc"
            return j + 1

        for ins in insts:
            si = ins.sync_info
            if not si:
                continue
            for w in si.on_wait:
                if w.ant_name == sem:
                    assert w.wait_mode == "sem-ge-imm", w.wait_mode
                    w.wait_value = new_thresh(w.wait_value)
        for j, (_, ins, u) in enumerate(us):
            if not kept[j]:
                si = ins.sync_info
                rest = [x for x in si.on_update if x.ant_name != sem]
                assert len(rest) == len(si.on_update) - 1
                if rest:
                    si.on_update[:] = rest
                else:
                    si.on_update.clear()
                changed += 1
    if verbose:
        print(f"prune_pe_incs: removed {changed} matmul sem-incs")
    return changed


def build(debug=False):
    nc = bacc.Bacc("TRN2", target_bir_lowering=False, debug=debug,
                   num_devices=1)

    qT_e = nc.dram_tensor("qT8", [H, S], f8, kind="ExternalInput")
    kT_e = nc.dram_tensor("kT8", [H, S], f8, kind="ExternalInput")
    vT_e = nc.dram_tensor("vT", [H, S], bf16, kind="ExternalInput")
    wq_e = nc.dram_tensor("wq8", [H, GH], f8, kind="ExternalInput")
    wk_e = nc.dram_tensor("wk8", [H, GH], f8, kind="ExternalInput")
    wv_e = nc.dram_tensor("wvb", [H, GH], bf16, kind="ExternalInput")
    wp_e = nc.dram_tensor("wpb", [GH, H], bf16, kind="ExternalInput")
    out_e = nc.dram_tensor("out", [S, H], f32, kind="ExternalOutput")

    with tile.TileContext(nc) as tc:
        with (
            tc.tile_pool(name="wts", bufs=1) as wts,
            tc.tile_pool(name="inb", bufs=2) as inb,
            tc.tile_pool(name="act", bufs=2) as actp,
            tc.tile_pool(name="scn", bufs=2) as scn,
            tc.tile_pool(name="sgp", bufs=2) as sgp,
            tc.tile_pool(name="yp", bufs=2) as yp,
            tc.tile_pool(name="tmp", bufs=3) as tmp,
            tc.tile_pool(name="osb", bufs=3) as osbp,
            tc.tile_pool(name="ps", bufs=3, space="PSUM") as ps,
            tc.tile_pool(name="pso", bufs=2, space="PSUM") as pso,
        ):
            # ---- weights (resident all kernel) -----------------------
            # Load order = first-use order: wk + first k chunk head the
            # DMA queues so the PE can start within a few us.
            def load_w(ext, name, dtype):
                t = wts.tile([128, KI if ext is not wp_e else MO,
                              GH if ext is not wp_e else H], dtype, tag=name)
                nsub = KI if ext is not wp_e else MO
                for kk in range(nsub):
                    nc.sync.dma_start(
                        t[:, kk, :], ext[kk * 128:(kk + 1) * 128, :])
                return t

            wk_t = load_w(wk_e, "wk", f8)

            def load_in(ext, tagpfx, dtype, s):
                t = inb.tile([128, KI, SC], dtype, tag=tagpfx)
                for kk in range(KI):
                    nc.sync.dma_start(
                        t[:, kk, :],
                        ext[kk * 128:(kk + 1) * 128, bass.ts(s, SC)])
                return t

            kc0 = load_in(kT_e, "ik", f8, 0)
            wv_t = load_w(wv_e, "wv", bf16)
            vc0 = load_in(vT_e, "iv", bf16, 0)
            wq_t = load_w(wq_e, "wq", f8)
            qc0 = load_in(qT_e, "iq", f8, 0)
            wp_t = load_w(wp_e, "wp", bf16)

            sk_prev = [None] * MO
            skv_prev = [None] * MO

            def kproj(kc, m):
                psm = ps.tile([128, SC], f32, tag="ps")
                for kk in range(KI2):
                    nc.tensor.matmul(
                        psm[:],
                        wk_t[:, 2 * kk:2 * kk + 2, m * 128:(m + 1) * 128],
                        kc[:, 2 * kk:2 * kk + 2, :],
                        start=(kk == 0), stop=(kk == KI2 - 1),
                        perf_mode=DR)
                t = actp.tile([128, SC], bf16, tag=f"ke{m}")
                nc.scalar.activation(t[:], psm[:], AF.Exp, scale=1.0 / WSCALE)
                return t

            def vproj(vc, m):
                psm = ps.tile([128, SC], f32, tag="ps")
                for kk in range(KI):
                    nc.tensor.matmul(
                        psm[:],
                        wv_t[:, kk, m * 128:(m + 1) * 128],
                        vc[:, kk, :],
                        start=(kk == 0), stop=(kk == KI - 1))
                return psm

            def qproj(qc, m):
                psm = ps.tile([128, SC], f32, tag="ps")
                for kk in range(KI2):
                    nc.tensor.matmul(
                        psm[:],
                        wq_t[:, 2 * kk:2 * kk + 2, m * 128:(m + 1) * 128],
                        qc[:, 2 * kk:2 * kk + 2, :],
                        start=(kk == 0), stop=(kk == KI2 - 1),
                        perf_mode=DR)
                t = sgp.tile([128, SC], bf16, tag=f"sg{m}")
                nc.scalar.activation(t[:], psm[:], AF.Sigmoid,
                                     scale=1.0 / WSCALE)
                return t

            def sk_scan(s, ke, m):
                skt = scn.tile([128, SC], f32, tag=f"sk{m}")
                init = 0.0 if s == 0 else sk_prev[m][:, SC - 1:SC]
                nc.vector.tensor_tensor_scan(
                    skt[:], ke[:], ke[:], init, OP.add, OP.bypass)
                sk_prev[m] = skt

            def kv_mul(ke, psm, m):
                kv = actp.tile([128, SC], bf16, tag=f"kv{m}")
                nc.vector.tensor_mul(kv[:], ke[:], psm[:])
                return kv

            def skv_scan(s, kv, m):
                skvt = scn.tile([128, SC], bf16, tag=f"sv{m}")
                init = 0.0 if s == 0 else skv_prev[m][:, SC - 1:SC]
                nc.vector.tensor_tensor_scan(
                    skvt[:], kv[:], kv[:], init, OP.add, OP.bypass)
                skv_prev[m] = skvt

            def ydve(state, ms, ys):
                """y = sg * skv / sk for the given m tiles (DVE)."""
                sg, sks, skvs = state
                for m in ms:
                    rcp = tmp.tile([128, SC], f32, tag="rcp")
                    nc.vector.reciprocal_approx_fast(rcp[:], sks[m][:])
                    rat = tmp.tile([128, SC], bf16, tag="rat")
                    nc.vector.tensor_mul(rat[:], skvs[m][:], rcp[:])
                    y = yp.tile([128, SC], bf16, tag=f"y{m}")
                    nc.vector.tensor_mul(y[:], rat[:], sg[m][:])
                    ys[m] = y

            def emit_chunk(s, prev_state, pre=None):
                """Emit one chunk's projections + scans, interleaved with
                the PREVIOUS chunk's y computation and out projection.

                DVE FIFO order is tuned against a ~90%-utilized DVE:
                kv muls must land early (they gate proj-PSUM recycling
                for the q projection), y(s-1) must be complete before
                the PE reaches the out-projection matmuls, and the
                scans — whose consumers are a chunk away — fill the
                remaining slack at the back.
                """
                if pre is None:
                    kc = load_in(kT_e, "ik", f8, s)
                    vc = load_in(vT_e, "iv", bf16, s)
                    qc = load_in(qT_e, "iq", f8, s)
                else:
                    kc, vc, qc = pre
                ys = [None] * MO
                ke = [kproj(kc, m) for m in range(MO)]
                # v projections emitted pairwise with their kv muls so a
                # ps-pool (bufs=3) buffer is never re-tagged before its
                # previous consumer exists in the stream.
                psv0 = vproj(vc, 0)
                psv1 = vproj(vc, 1)
                if prev_state is not None:
                    ydve(prev_state, (0, 1), ys)
                sk_scan(s, ke[0][:], 0)
                sk_scan(s, ke[1][:], 1)
                kv0 = kv_mul(ke[0][:], psv0[:], 0)
                skv_scan(s, kv0[:], 0)
                kv1 = kv_mul(ke[1][:], psv1[:], 1)
                skv_scan(s, kv1[:], 1)
                psv2 = vproj(vc, 2)
                kv2 = kv_mul(ke[2][:], psv2[:], 2)
                psv3 = vproj(vc, 3)
                kv3 = kv_mul(ke[3][:], psv3[:], 3)
                if prev_state is not None:
                    ydve(prev_state, (2, 3), ys)
                sk_scan(s, ke[2][:], 2)
                sk_scan(s, ke[3][:], 3)
                skv_scan(s, kv2[:], 2)
                skv_scan(s, kv3[:], 3)
                sg = [qproj(qc, m) for m in range(MO)]
                state = (sg, [sk_prev[m] for m in range(MO)],
                         [skv_prev[m] for m in range(MO)])
                return state, ys

            def emit_out(s, ys):
                for t4 in range(NT):
                    psm = pso.tile([128, H], f32)
                    tsl = bass.ts(t4, 128)
                    for m in range(MO):
                        for n in range(2):
                            nc.tensor.matmul(
                                psm[:, n * 512:(n + 1) * 512],
                                ys[m][:, tsl],
                                wp_t[:, m, n * 512:(n + 1) * 512],
                                start=(m == 0), stop=(m == MO - 1))
                    ob = osbp.tile([128, H], f32, tag="ob")
                    nc.scalar.copy(ob[:], psm[:])
                    nc.sync.dma_start(
                        out_e[s * SC + t4 * 128:s * SC + (t4 + 1) * 128, :],
                        ob[:])

            # software pipeline: out projection rides one chunk behind
            state, _ = emit_chunk(0, None, pre=(kc0, vc0, qc0))
            for s in range(1, NCH):
                state, ys = emit_chunk(s, state)
                emit_out(s - 1, ys)
            ys = [None] * MO
            ydve(state, (0, 1, 2, 3), ys)
            emit_out(NCH - 1, ys)

    nc.compile()
    prune_pe_incs(nc, verbose=True)
    return nc


def _in_maps(q, k, v, w_q, w_k, w_v, w_p):
    bf = ml_dtypes.bfloat16
    e4 = ml_dtypes.float8_e4m3
    per_b = []
    for b in range(B):
        per_b.append({
            "qT8": np.ascontiguousarray(q[b].T).astype(e4),
            "kT8": np.ascontiguousarray(k[b].T).astype(e4),
            "vT": np.ascontiguousarray(v[b].T).astype(bf),
        })
    per_g = []
    for g in range(2):
        sl = slice(g * GH, (g + 1) * GH)
        per_g.append({
            "wq8": (w_q[:, sl] * WSCALE).astype(e4),
            "wk8": (w_k[:, sl] * WSCALE).astype(e4),
            "wvb": w_v[:, sl].astype(bf),
            "wpb": w_p[sl, :].astype(bf),
        })
    return [{**per_b[c // 2], **per_g[c % 2]} for c in range(NCORES)]


def run(q, k, v, w_q, w_k, w_v, w_p, trace=False, tmpdir=None):
    if "nc" not in _cache:
        _cache["nc"] = build()
    nc = _cache["nc"]
    in_maps = _in_maps(q, k, v, w_q, w_k, w_v, w_p)
    res = run_bass_kernel_spmd(nc, in_maps, core_ids=list(range(NCORES)),
                               trace=trace, tmpdir=tmpdir)
    out = np.empty((B, S, H), np.float32)
    for b in range(B):
        out[b] = res.results[2 * b]["out"]
        out[b] += res.results[2 * b + 1]["out"]
    return out, res


def kernel(**inputs):
    out, _ = run(**{k: np.asarray(v) for k, v in inputs.items()})
    return out
